# revision 34
# baseline (speedup 1.0000x reference)
"""Trainium2 Bass kernel for nn_AlignmentMatrix.

score[b,i,j] = [body_i ; pun_j ; body_i*pun_j] @ w_u
            = (body @ Bhat^T)[i,j] + s_pun[j]
where Bhat[j,d] = w3[d]*pun[j,d] + w1[d]  (folds s_cross and s_body)
and   s_pun[j] = sum_d w2[d]*pun[j,d]     (folded in via an extra
      PSUM-accumulating matmul with a [d,i]-replicated w2 stationary).

Sharding: data-parallel over batch across 8 NeuronCores (8 batches/core).
"""

import numpy as np

B, L, D = 64, 1024, 128
N_CORES = 8
BPC = B // N_CORES  # batches per core
P = 128
JT = 512  # matmul moving free dim (one PSUM bank of fp32)

_CACHE = {}

# Best configuration found via cost-model (TimelineSim) sweeps: contiguous
# "flat" body loads (store AP undoes the row permutation), paired 1MB stores
# alternating between the SP and ACT HWDGE rings, loads issued from the
# otherwise-idle GpSimd SWDGE, deeper staging pools.
DEFAULT_TUNE = {
    "flat_load": True,
    "flat_pun": True,
    "pair_stores": True,
    "pair_loads": True,
    "store_engines": ["sync", "scalar"],
    "load_engine": "gpsimd",
    "nat": 4,
    "outs": 12,
    "tt": 3,
    "dve_share": 3,
}


def _build(bpc=BPC, repeats=1, tune=None):
    from contextlib import ExitStack

    import concourse.tile as tile
    from concourse import bacc, mybir
    from concourse.masks import make_identity

    tune = dict(DEFAULT_TUNE if tune is None else tune)
    NAT_BUFS = tune.get("nat", 2)
    TT_BUFS = tune.get("tt", 2)
    OUT_BUFS = tune.get("outs", 6)
    TR_PS_BUFS = tune.get("tr_ps", 2)
    MM_PS_BUFS = tune.get("mm_ps", 3)
    DVE_SHARE = tune.get("dve_share", 2)  # it % 2 < dve_share -> DVE copy

    f32 = mybir.dt.float32
    f32r = mybir.dt.float32r
    Identity = mybir.ActivationFunctionType.Identity

    nc = bacc.Bacc("TRN2", target_bir_lowering=False, debug=False, num_devices=N_CORES)

    body = nc.dram_tensor("body", [bpc, L, D], f32, kind="ExternalInput").ap()
    pun = nc.dram_tensor("pun", [bpc, L, D], f32, kind="ExternalInput").ap()
    w_u = nc.dram_tensor("w_u", [3 * D, 1], f32, kind="ExternalInput").ap()
    PROXY = tune.pop("proxy", False)
    if PROXY:
        # Timing-proxy mode: identical instruction stream, but the big output
        # lands in internal DRAM (no host transfer); a tiny external output is
        # chained through every repeat so host-visible completion covers all
        # the work.
        out = nc.dram_tensor("oscratch", [bpc, L, L], f32).ap()
        outx = nc.dram_tensor("out", [P, P], f32, kind="ExternalOutput").ap()
    else:
        out = nc.dram_tensor("out", [bpc, L, L], f32, kind="ExternalOutput").ap()

    with tile.TileContext(nc) as tc, ExitStack() as ctx:
        consts = ctx.enter_context(tc.tile_pool(name="consts", bufs=1))
        nat_pool = ctx.enter_context(tc.tile_pool(name="nat", bufs=NAT_BUFS))
        att_pool = ctx.enter_context(tc.tile_pool(name="att", bufs=TT_BUFS))
        btt_pool = ctx.enter_context(tc.tile_pool(name="btt", bufs=TT_BUFS))
        bht_pool = ctx.enter_context(tc.tile_pool(name="bht", bufs=TT_BUFS))
        out_pool = ctx.enter_context(tc.tile_pool(name="outs", bufs=OUT_BUFS))
        tr_ps = ctx.enter_context(
            tc.tile_pool(name="tr_ps", bufs=TR_PS_BUFS, space="PSUM")
        )
        mm_ps = ctx.enter_context(
            tc.tile_pool(name="mm_ps", bufs=MM_PS_BUFS, space="PSUM")
        )

        FLAT = tune.get("flat_load", False)
        PAIR = tune.get("pair_stores", False)
        DEFAULT_LOAD_ENG = {"sync": nc.sync, "gpsimd": nc.gpsimd, "scalar": nc.scalar}[
            tune.get("load_engine", "sync")
        ]
        STORE_ENGS = [
            {"sync": nc.sync, "gpsimd": nc.gpsimd, "scalar": nc.scalar}[e]
            for e in tune.get("store_engines", ["sync"])
        ]

        PAIR_LOADS = tune.get("pair_loads", False)
        FLAT_PUN = tune.get("flat_pun", False)

        def issue_loads(b, eng=None):
            # With FLAT loads, DRAM reads are fully contiguous per partition and
            # free-slice t of the nat tile holds rows {8q + t}; i-ordering within
            # each transposed group is permuted, undone by the store AP below.
            # With PAIR_LOADS, two batches ride one 1MB DMA; slices are returned
            # per batch.
            nb = 2 if PAIR_LOADS else 1
            natb = nat_pool.tile([P, nb, 8, P], f32, tag="natb")
            natp = nat_pool.tile([P, nb, 8, P], f32, tag="natp")
            bsl = slice(b, b + nb)
            LOAD_ENG = eng if eng is not None else DEFAULT_LOAD_ENG
            if FLAT:
                LOAD_ENG.dma_start(
                    natb[:], body[bsl].rearrange("b2 (p t) d -> p b2 t d", t=8)
                )
            else:
                LOAD_ENG.dma_start(
                    natb[:], body[bsl].rearrange("b2 (t p) d -> p b2 t d", p=P)
                )
            if FLAT_PUN:
                # Contiguous load; matmul output columns come out in the
                # permuted order j = 8x + t, undone by the permuted-AP copy
                # on PSUM->SBUF eviction below.
                LOAD_ENG.dma_start(
                    natp[:], pun[bsl].rearrange("b2 (p t) d -> p b2 t d", t=8)
                )
            else:
                LOAD_ENG.dma_start(
                    natp[:], pun[bsl].rearrange("b2 (t p) d -> p b2 t d", p=P)
                )
            return natb, natp

        order = [b for _ in range(repeats) for b in range(bpc)]
        if PAIR_LOADS:
            assert bpc % 2 == 0
        # Issue the first loads before the constants preamble so the DMA
        # engines start moving bytes immediately (make_identity otherwise
        # blocks the Pool sequencer that also issues the loads). Use the
        # HWDGE sync ring here: its first-byte latency beats SWDGE's.
        hoisted = {0: issue_loads(order[0], eng=nc.sync)}

        identity = consts.tile([P, P], f32)
        make_identity(nc, identity[:])

        # w_u columns: wcols[:, k] = w_u[k*128:(k+1)*128, 0]; k=0 -> w1, 1 -> w2, 2 -> w3
        wcols = consts.tile([P, 3], f32)
        nc.sync.dma_start(wcols[:], w_u.rearrange("(k p) one -> p (k one)", p=P))

        # W2_rep[d, i] = w2[d] for all i (stationary operand broadcasting s_pun)
        zeros = consts.tile([P, P], f32)
        nc.vector.memset(zeros[:], 0.0)
        w2rep = consts.tile([P, P], f32r)
        nc.scalar.activation(w2rep[:], zeros[:], Identity, bias=wcols[:, 1:2])
        if PROXY:
            sink = consts.tile([P, 512], f32)
            nc.vector.memset(sink[:], 0.0)

        for idx, b in enumerate(order):
            if PAIR_LOADS:
                if b % 2 == 0:
                    natbp, natpp = hoisted.pop(idx, None) or issue_loads(b)
                natb, natp = natbp[:, b % 2], natpp[:, b % 2]
            else:
                natb, natp = hoisted.pop(idx, None) or issue_loads(b)
                natb, natp = natb[:, 0], natp[:, 0]

            # Transpose body -> AT [d, i], pun -> BT [d, j]; 4 PE transposes per
            # PSUM bank, then one ScalarE copy (casts to f32r) back to SBUF.
            AT = att_pool.tile([P, L], f32r)
            BT = btt_pool.tile([P, L], f32r)
            for nat, dst in ((natb, AT), (natp, BT)):
                for g in range(2):
                    pst = tr_ps.tile([P, JT], f32, tag="trps")
                    for t4 in range(4):
                        t = g * 4 + t4
                        nc.tensor.transpose(
                            pst[:, t4 * P : (t4 + 1) * P], nat[:, t, :], identity[:]
                        )
                    nc.scalar.copy(dst[:, g * JT : (g + 1) * JT], pst[:])

            # Bhat^T = w3[d]*BT + w1[d]
            BH = bht_pool.tile([P, L], f32r)
            nc.scalar.activation(
                BH[:], BT[:], Identity, bias=wcols[:, 0:1], scale=wcols[:, 2:3]
            )

            if FLAT:
                # row for (tile it, psum partition q) is 8q + it
                out_r = out.rearrange("bb (q e) d -> bb q e d", e=8)
            n_store = 0
            ot = None
            for it in range(8):
                pmm = mm_ps.tile([P, L], f32)
                for jh in range(2):
                    js = slice(jh * JT, (jh + 1) * JT)
                    nc.tensor.matmul(
                        pmm[:, js], w2rep[:], BT[:, js], start=True, stop=False
                    )
                    nc.tensor.matmul(
                        pmm[:, js],
                        AT[:, it * P : (it + 1) * P],
                        BH[:, js],
                        start=False,
                        stop=True,
                    )
                if PAIR:
                    if it % 2 == 0:
                        ot = out_pool.tile([P, 2, L], f32)
                    half = ot[:, it % 2, :]
                else:
                    ot = out_pool.tile([P, L], f32)
                    half = ot[:]
                if FLAT_PUN:
                    # psum column c = t*128 + x holds j = 8x + t; write in
                    # natural j order via matching 2D free APs.
                    csrc = pmm[:].rearrange("p (t x) -> p x t", x=P)
                    cdst = half.rearrange("p (x t) -> p x t", t=8)
                else:
                    csrc, cdst = pmm[:], half
                if it % 4 < DVE_SHARE:
                    nc.vector.tensor_copy(cdst, csrc)
                else:
                    nc.scalar.copy(cdst, csrc)
                if PAIR and it % 2 == 0:
                    continue
                eng = STORE_ENGS[n_store % len(STORE_ENGS)]
                n_store += 1
                if PAIR:
                    it0 = it - 1
                    if FLAT:
                        dst = out_r[b, :, it0 : it0 + 2, :]
                    else:
                        dst = out[b, it0 * P : (it0 + 2) * P, :].rearrange(
                            "(e q) d -> q e d", e=2
                        )
                    eng.dma_start(dst, ot[:])
                else:
                    if FLAT:
                        dst = out_r[b, :, it, :]
                    else:
                        dst = out[b, it * P : (it + 1) * P, :]
                    eng.dma_start(dst, ot[:])

            if PROXY and b == bpc - 1:
                rb = consts.tile([P, 512], f32, tag="rb")
                nc.sync.dma_start(rb[:], out[b, :P, :512])
                nc.vector.tensor_add(sink[:], sink[:], rb[:])

        if PROXY:
            fin = consts.tile([P, P], f32, tag="fin")
            nc.vector.tensor_copy(fin[:], sink[:, :P])
            nc.sync.dma_start(outx[:], fin[:])

    nc.compile()
    return nc


def get_nc(bpc=BPC, repeats=1, tune=None):
    key = (bpc, repeats, tuple(sorted((tune or {}).items())))
    if key not in _CACHE:
        _CACHE[key] = _build(bpc, repeats, tune)
    return _CACHE[key]


def _make_runner(nc):
    """Reusable sharded-jit executor for the compiled Bass program (mirrors
    concourse.bass2jax.run_bass_via_pjrt, but built once and reused so repeat
    kernel() calls skip re-lowering/recompiling)."""
    import jax
    from jax.experimental.shard_map import shard_map
    from jax.sharding import Mesh, PartitionSpec

    from concourse import mybir
    from concourse.bass2jax import (
        _bass_exec_p,
        install_neuronx_cc_hook,
        partition_id_tensor,
    )

    install_neuronx_cc_hook()

    partition_name = nc.partition_id_tensor.name if nc.partition_id_tensor else None
    in_names, out_names, out_avals, zero_shapes = [], [], [], []
    for alloc in nc.m.functions[0].allocations:
        if not isinstance(alloc, mybir.MemoryLocationSet):
            continue
        name = alloc.memorylocations[0].name
        if alloc.kind == "ExternalInput":
            if name != partition_name:
                in_names.append(name)
        elif alloc.kind == "ExternalOutput":
            out_names.append(name)
            shape = tuple(alloc.tensor_shape)
            dtype = mybir.dt.np(alloc.dtype)
            out_avals.append(jax.core.ShapedArray(shape, dtype))
            zero_shapes.append((shape, dtype))
    n_params = len(in_names)
    n_outs = len(out_avals)
    all_in_names = list(in_names) + out_names
    if partition_name is not None:
        all_in_names.append(partition_name)
    donate = tuple(range(n_params, n_params + n_outs))

    def _body(*args):
        operands = list(args)
        if partition_name is not None:
            operands.append(partition_id_tensor())
        outs = _bass_exec_p.bind(
            *operands,
            out_avals=tuple(out_avals),
            in_names=tuple(all_in_names),
            out_names=tuple(out_names),
            lowering_input_output_aliases=(),
            sim_require_finite=True,
            sim_require_nnan=True,
            nc=nc,
        )
        return tuple(outs)

    devices = jax.devices()[:N_CORES]
    mesh = Mesh(np.asarray(devices), ("core",))
    in_specs = (PartitionSpec("core"),) * (n_params + n_outs)
    out_specs = (PartitionSpec("core"),) * len(out_names)
    sharded = jax.jit(
        shard_map(
            _body, mesh=mesh, in_specs=in_specs, out_specs=out_specs, check_rep=False
        ),
        donate_argnums=donate,
        keep_unused=True,
    )

    import jax.numpy as jnp
    from jax.sharding import NamedSharding

    # Donated output buffers are zero-filled ON DEVICE — avoids shipping
    # 256MB of host zeros through the tunnel on every call.
    zeros_fn = jax.jit(
        lambda: tuple(
            jnp.zeros((N_CORES * s[0], *s[1:]), dt) for s, dt in zero_shapes
        ),
        out_shardings=tuple(
            NamedSharding(mesh, PartitionSpec("core")) for _ in zero_shapes
        ),
    )

    def run(in_maps):
        concat_in = [
            np.concatenate(
                [np.asarray(in_maps[c][name]) for c in range(N_CORES)], axis=0
            )
            for name in in_names
        ]
        out_arrs = sharded(*concat_in, *zeros_fn())
        return [
            {
                name: np.asarray(out_arrs[i]).reshape(
                    N_CORES, *out_avals[i].shape
                )[c]
                for i, name in enumerate(out_names)
            }
            for c in range(N_CORES)
        ]

    return run


def kernel(batch_size=None, body=None, pun=None, w_u=None, **_):
    if "runner" not in _CACHE:
        _CACHE["runner"] = _make_runner(get_nc())
    body = np.ascontiguousarray(body, dtype=np.float32)
    pun = np.ascontiguousarray(pun, dtype=np.float32)
    w_u = np.ascontiguousarray(w_u, dtype=np.float32).reshape(3 * D, 1)
    in_maps = [
        {
            "body": body[c * BPC : (c + 1) * BPC],
            "pun": pun[c * BPC : (c + 1) * BPC],
            "w_u": w_u,
        }
        for c in range(N_CORES)
    ]
    results = _CACHE["runner"](in_maps)
    return np.concatenate([results[c]["out"] for c in range(N_CORES)], axis=0)
